# revision 29
# baseline (speedup 1.0000x reference)
"""Trainium2 Bass kernel for nn_DiscreteAutoregressiveFlow (sampling, forward).

Math: `inputs` is an exact one-hot [B, L, V] tensor. For a row holding token v:
  net = W[v] + b                      (exact: one-hot @ W picks a row)
  loc = one_hot(argmax(net[:V]));  scale = one_hot(argmax(net[V:]))
  one_hot_multiply -> one-hot at (scale_tok*v) % V   (zero row if scale_tok==0)
  one_hot_add      -> one-hot at (scale_tok*v + loc_tok) % V
So out[row] = one_hot(cmap[v]) with a host-precomputed 64-entry map
(sentinel >= V encodes the zero row). The straight-through softmax residuals
and FFT noise in the reference are O(1e-7) and vanish in norm relative error.

Device pipeline per contiguous 128x(r*64) chunk (streaming, DVE-bound):
  xt   = DMA-in (sync HWDGE)
  xb   = cast f32->bf16            (scalar ACT; chunks 0-1 skip this and
                                    run a 1x f32 add straight off the DMA
                                    tile -- nothing gates the first add)
  prod = xb + cmap_f               (DVE TT bf16 2x, materialized table)
  m    = reduce_max(prod, inner V) (DVE 1x) = 1 + cmap[tok]/128, exact
  m2   = m duplicated into pairs   (tiny DVE copy)
  ybf  = is_equal(iota_f, m2)      (DVE TT 2x! pair-readable operand
                                    unlocks 2x_1P; bf16 out, exact 0/1)
  yf   = upcast bf16->f32          (scalar ACT; scalar has slack)
  DMA-out (sync HWDGE)
Orchestration learned from traces:
  - Tile serializes per-tile accesses by GLOBAL issue order: producers must
    be issued before consumers. The scalar/vector issue streams are woven.
  - sync FIFO: [in0, consts, in1..inN, out0..outN] -- no out-DMA can
    head-of-line-block the x stream.
  - scalar FIFO: split table broadcasts (16-row slice first), in-casts kept
    ahead of the eq-gated upcasts.
  - DVE FIFO: pair+eq+upcast of chunk i trail add/reduce of chunk i+1.
  - Chunk sizes ramp 8..32..4: small first chunks cut the head (~2us),
    big middle chunks amortize per-instruction overhead, small last chunks
    keep the eq->upcast->DMA-out drain short.
  - Measured DVE cost model: TT (151+FD/Accel)/0.96ns, reduce (141+FD)/0.96
    (1x only), Accel=2 for all-bf16 step-1 operands (incl. [m,m] pairs).
All f32/bf16 values involved are exact (c <= 127 and 2^-7 scaling).
Sharding: pure data parallel over B*L rows, 8 cores, no collectives.
"""

import numpy as np

V = 64
P = 128
N_CORES = 8
B, L = 16, 8192
ROWS = B * L                      # 131072
ROWS_PER_CORE = ROWS // N_CORES   # 16384
SENTINEL = 100.0
EPS = 1.0 / 128.0

# rows-per-partition per contiguous chunk; sum * 128 = rows per core.
R_LIST = (8, 16, 32, 32, 24, 12, 4)

_CACHE = {}


def _build_nc(rows_per_core: int, r_list):
    import concourse.bacc as bacc
    import concourse.mybir as mybir
    from concourse.bass import broadcast_tensor_aps
    from concourse.tile import TileContext

    f32 = mybir.dt.float32
    bf16 = mybir.dt.bfloat16
    n_chunks = len(r_list)
    r_max = max(r_list)
    assert rows_per_core == P * sum(r_list)
    row_off = [0]
    for r in r_list:
        row_off.append(row_off[-1] + P * r)

    # Bacc (not raw Bass): its compile() runs generate_event_semaphores(),
    # which legalizes multi-wait instructions for TRN2 (1 wait per instr).
    nc = bacc.Bacc("TRN2", target_bir_lowering=False, name="daf_onehot")
    x = nc.dram_tensor("x", [rows_per_core, V], f32, kind="ExternalInput")
    cmap = nc.dram_tensor("cmap", [P, V], f32, kind="ExternalInput")
    iota = nc.dram_tensor("iota", [P, V], f32, kind="ExternalInput")
    y = nc.dram_tensor("y", [rows_per_core, V], f32, kind="ExternalOutput")

    def chunk_view(t, ci, r):
        # rows [row_off[ci], row_off[ci]+128*r) as [128, r*64], partition-
        # contiguous: partition p holds r consecutive rows (4KB*r block).
        seg = t[row_off[ci] : row_off[ci + 1]]
        return seg.rearrange("(p r) v -> p (r v)", p=P, r=r)

    with TileContext(nc) as tc:
        with (
            tc.tile_pool(name="const", bufs=1) as constp,
            tc.tile_pool(name="io", bufs=1) as iop,
            tc.tile_pool(name="work", bufs=1) as workp,
        ):
            cmap_st = constp.tile([P, V], f32, tag="cmap_st")
            iota_st = constp.tile([P, V], f32, tag="iota_st")

            # sync FIFO: chunk 0's x first, tiny consts next, rest of x.
            xts = []
            for ci, r in enumerate(r_list):
                xt = iop.tile([P, r * V], f32, tag=f"x{ci}")
                nc.sync.dma_start(xt[:], chunk_view(x, ci, r))
                xts.append(xt)
                if ci == 0:
                    nc.sync.dma_start(cmap_st[:], cmap[:])
                    nc.sync.dma_start(iota_st[:], iota[:])

            cmap_1 = cmap_st[:].rearrange("p (o v) -> p o v", o=1)
            iota_1 = iota_st[:].rearrange("p (o v) -> p o v", o=1)
            # Table materialization is split: a 16-row slice unblocks the
            # first two (small) chunks fast; the full-size tables are built
            # while the DVE chews on those chunks.
            r_bc = min(16, r_max)
            cmap_f = constp.tile([P, r_max * V], bf16, tag="cmap_f")
            iota_f = constp.tile([P, r_max * V], bf16, tag="iota_f")

            def bcast(dst_tile, src1, lo, hi):
                d3 = dst_tile[:, lo * V : hi * V].rearrange(
                    "p (r v) -> p r v", v=V
                )
                s_b, _ = broadcast_tensor_aps(src1, d3)
                nc.scalar.copy(d3, s_b)

            # Per-chunk op lambdas; each engine's FIFO order is chosen
            # explicitly below (Tile orders by issue-call order per engine).
            adds, reds, pairs, eqs = [], [], [], []
            in_casts, up_casts = {}, []
            yfs = []
            N_F32 = 2   # chunks taking the cast-free f32 add (head chunks)

            def make_chunk(ci):
                r = r_list[ci]
                fd = r * V
                prod = workp.tile([P, fd], bf16, tag=f"prod{ci}")
                p3 = prod[:].rearrange("p (r v) -> p r v", v=V)
                if ci < N_F32:
                    # f32 add straight off the DMA tile + broadcast f32 cmap:
                    # 1x mode, but waits on nothing except the x data. Kills
                    # the in-cast and table dependency on the critical head.
                    x3 = xts[ci][:].rearrange("p (r v) -> p r v", v=V)
                    cmf_b, _ = broadcast_tensor_aps(cmap_1, x3)
                    adds.append(
                        lambda p3=p3, x3=x3, cmf_b=cmf_b: nc.vector.tensor_tensor(
                            p3, x3, cmf_b, op=mybir.AluOpType.add
                        )
                    )
                else:
                    xb_d = workp.tile([P, fd], bf16, tag=f"xb{ci}")
                    in_casts[ci] = lambda xb_d=xb_d, ci=ci: nc.scalar.copy(
                        xb_d[:], xts[ci][:]
                    )
                    adds.append(
                        lambda prod=prod, xb_d=xb_d, fd=fd: nc.vector.tensor_tensor(
                            prod[:], xb_d[:], cmap_f[:, :fd],
                            op=mybir.AluOpType.add,
                        )
                    )
                c_t = workp.tile([P, r], f32, tag=f"c{ci}")
                reds.append(
                    lambda c_t=c_t, p3=p3: nc.vector.tensor_reduce(
                        c_t[:], p3, axis=mybir.AxisListType.X,
                        op=mybir.AluOpType.max,
                    )
                )
                # Duplicate m into adjacent pairs: the is_equal's second
                # operand becomes pair-readable, unlocking TT 2x_1P mode
                # (a plain stride-0 broadcast operand is stuck at 1x).
                m2 = workp.tile([P, 2 * r], bf16, tag=f"m2{ci}")
                m2_3 = m2[:].rearrange("p (r two) -> p r two", two=2)
                c3 = c_t[:].rearrange("p (r one) -> p r one", one=1)
                c3_b, _ = broadcast_tensor_aps(c3, m2_3)
                pairs.append(
                    lambda m2_3=m2_3, c3_b=c3_b: nc.vector.tensor_copy(m2_3, c3_b)
                )
                y_bf = workp.tile([P, fd], bf16, tag=f"ybf{ci}")
                o4 = y_bf[:].rearrange("p (r k two) -> p r k two", two=2, k=V // 2)
                i4 = iota_f[:, :fd].rearrange(
                    "p (r k two) -> p r k two", two=2, k=V // 2
                )
                m2_4 = m2_3.rearrange("p r (one two) -> p r one two", one=1)
                m2_b, _ = broadcast_tensor_aps(m2_4, o4)
                eqs.append(
                    lambda o4=o4, i4=i4, m2_b=m2_b: nc.vector.tensor_tensor(
                        o4, i4, m2_b, op=mybir.AluOpType.is_equal
                    )
                )
                # f32 upcast on scalar (it has slack); out-DMA reads this.
                yf = iop.tile([P, fd], f32, tag=f"yf{ci}")
                up_casts.append(
                    lambda yf=yf, y_bf=y_bf: nc.scalar.copy(yf[:], y_bf[:])
                )
                yfs.append(yf)

            for ci in range(n_chunks):
                make_chunk(ci)

            # Issue order is load-bearing twice over: Tile serializes
            # accesses to a tile by GLOBAL issue order (producers must be
            # issued before consumers), and each engine's FIFO executes in
            # its own issue order. Weave scalar and vector issues so both
            # hold: tables + early in-casts first, then per chunk the DVE
            # chain (pair+eq trailing one chunk) with each upcast issued
            # right after its eq and each later in-cast ahead of its add.
            bcast(cmap_f, cmap_1, 0, r_bc)
            if r_max > r_bc:
                bcast(cmap_f, cmap_1, r_bc, r_max)
            bcast(iota_f, iota_1, 0, r_bc)
            if 2 in in_casts:
                in_casts[2]()
            if r_max > r_bc:
                bcast(iota_f, iota_1, r_bc, r_max)
            if 3 in in_casts:
                in_casts[3]()

            adds[0]()
            reds[0]()
            adds[1]()
            reds[1]()
            for ci in range(2, n_chunks):
                pairs[ci - 2]()
                eqs[ci - 2]()
                up_casts[ci - 2]()
                if ci + 2 in in_casts:
                    in_casts[ci + 2]()
                adds[ci]()
                reds[ci]()
            for ci in range(n_chunks - 2, n_chunks):
                pairs[ci]()
                eqs[ci]()
                up_casts[ci]()

            for ci, r in enumerate(r_list):
                nc.sync.dma_start(chunk_view(y, ci, r), yfs[ci][:])

    # Bacc.finalize runs compile(): wait-splitting (generate_event_semaphores),
    # register allocation, nop fusion. run_bass_via_pjrt serializes nc.m as-is,
    # so this must happen here.
    nc.finalize()
    return nc


def _get_nc(rows_per_core=ROWS_PER_CORE, r_list=R_LIST):
    key = (rows_per_core, tuple(r_list))
    if key not in _CACHE:
        _CACHE[key] = _build_nc(rows_per_core, r_list)
    return _CACHE[key]


def _host_cmap(W: np.ndarray, b: np.ndarray) -> np.ndarray:
    """64-entry map token -> output one-hot index (or sentinel for zero row)."""
    net = W.astype(np.float32) + b.astype(np.float32)[None, :]   # [V, 2V]
    loc_tok = np.argmax(net[:, :V], axis=1)                      # [V]
    scale_tok = np.argmax(net[:, V:], axis=1)                    # [V]
    t = (scale_tok * np.arange(V, dtype=np.int64) + loc_tok) % V
    return np.where(scale_tok == 0, SENTINEL, t.astype(np.float64)).astype(
        np.float32
    )


def _host_tables(W: np.ndarray, b: np.ndarray):
    cmap_eps = _host_cmap(W, b) * np.float32(EPS)                  # exact f32
    iota_eps = 1.0 + np.arange(V, dtype=np.float32) * np.float32(EPS)
    cmap_t = np.tile(cmap_eps.astype(np.float32)[None, :], (P, 1))
    iota_t = np.tile(iota_eps.astype(np.float32)[None, :], (P, 1))
    return cmap_t, iota_t


def kernel(inputs: np.ndarray, W: np.ndarray, b: np.ndarray) -> np.ndarray:
    from concourse import bass_utils

    x = np.ascontiguousarray(inputs.astype(np.float32, copy=False).reshape(ROWS, V))
    cmap_t, iota_t = _host_tables(W, b)

    nc = _get_nc()
    in_maps = [
        {
            "x": x[c * ROWS_PER_CORE : (c + 1) * ROWS_PER_CORE],
            "cmap": cmap_t,
            "iota": iota_t,
        }
        for c in range(N_CORES)
    ]
    res = bass_utils.run_bass_kernel_spmd(nc, in_maps, core_ids=list(range(N_CORES)))
    y = np.concatenate([r["y"] for r in res.results], axis=0)
    return y.reshape(inputs.shape).astype(inputs.dtype, copy=False)


# revision 33
# speedup vs baseline: 1.0510x; 1.0510x over previous
"""Trainium2 Bass kernel for nn_DiscreteAutoregressiveFlow (sampling, forward).

Math: `inputs` is an exact one-hot [B, L, V] tensor. For a row holding token v:
  net = W[v] + b                      (exact: one-hot @ W picks a row)
  loc = one_hot(argmax(net[:V]));  scale = one_hot(argmax(net[V:]))
  one_hot_multiply -> one-hot at (scale_tok*v) % V   (zero row if scale_tok==0)
  one_hot_add      -> one-hot at (scale_tok*v + loc_tok) % V
So out[row] = one_hot(cmap[v]) with a host-precomputed 64-entry map
(sentinel >= V encodes the zero row). The straight-through softmax residuals
and FFT noise in the reference are O(1e-7) and vanish in norm relative error.

Device pipeline per contiguous 128x(r*64) chunk (streaming, DVE-bound):
  xt   = DMA-in (sync HWDGE)
  xb   = cast f32->bf16            (scalar ACT; chunks 0-1 skip this and
                                    run a 1x f32 add straight off the DMA
                                    tile -- nothing gates the first add)
  prod = xb + cmap_f               (DVE TT bf16 2x, materialized table)
  m    = reduce_max(prod, inner V) (DVE 1x) = 1 + cmap[tok]/128, exact
  m2   = m duplicated into pairs   (tiny DVE copy)
  ybf  = is_equal(iota_f, m2)      (DVE TT 2x! pair-readable operand
                                    unlocks 2x_1P; bf16 out, exact 0/1)
  yf   = upcast bf16->f32          (scalar ACT; scalar has slack)
  DMA-out (sync HWDGE)
Orchestration learned from traces:
  - Tile serializes per-tile accesses by GLOBAL issue order: producers must
    be issued before consumers. The scalar/vector issue streams are woven.
  - sync FIFO: [in0, consts, in1..inN, out0..outN] -- no out-DMA can
    head-of-line-block the x stream.
  - scalar FIFO: split table broadcasts (16-row slice first), in-casts kept
    ahead of the eq-gated upcasts.
  - DVE FIFO: pair+eq+upcast of chunk i trail add/reduce of chunk i+1.
  - Chunk sizes ramp 8..32..4: small first chunks cut the head (~2us),
    big middle chunks amortize per-instruction overhead, small last chunks
    keep the eq->upcast->DMA-out drain short.
  - Measured DVE cost model: TT (151+FD/Accel)/0.96ns, reduce (141+FD)/0.96
    (1x only), Accel=2 for all-bf16 step-1 operands (incl. [m,m] pairs).
All f32/bf16 values involved are exact (c <= 127 and 2^-7 scaling).
Sharding: pure data parallel over B*L rows, 8 cores, no collectives.
"""

import numpy as np

V = 64
P = 128
N_CORES = 8
B, L = 16, 8192
ROWS = B * L                      # 131072
ROWS_PER_CORE = ROWS // N_CORES   # 16384
SENTINEL = 100.0
EPS = 1.0 / 128.0

# rows-per-partition per contiguous chunk; sum * 128 = rows per core.
R_LIST = (8, 16, 32, 32, 24, 12, 4)

_CACHE = {}


def _build_nc(rows_per_core: int, r_list):
    import concourse.bacc as bacc
    import concourse.mybir as mybir
    from concourse.bass import broadcast_tensor_aps
    from concourse.tile import TileContext

    f32 = mybir.dt.float32
    bf16 = mybir.dt.bfloat16
    n_chunks = len(r_list)
    r_max = max(r_list)
    assert rows_per_core == P * sum(r_list)
    row_off = [0]
    for r in r_list:
        row_off.append(row_off[-1] + P * r)

    # Bacc (not raw Bass): its compile() runs generate_event_semaphores(),
    # which legalizes multi-wait instructions for TRN2 (1 wait per instr).
    nc = bacc.Bacc("TRN2", target_bir_lowering=False, name="daf_onehot")
    x = nc.dram_tensor("x", [rows_per_core, V], f32, kind="ExternalInput")
    cmap = nc.dram_tensor("cmap", [P, V], f32, kind="ExternalInput")
    iota = nc.dram_tensor("iota", [P, V], f32, kind="ExternalInput")
    y = nc.dram_tensor("y", [rows_per_core, V], f32, kind="ExternalOutput")

    def chunk_view(t, ci, r):
        # rows [row_off[ci], row_off[ci]+128*r) as [128, r*64], partition-
        # contiguous: partition p holds r consecutive rows (4KB*r block).
        seg = t[row_off[ci] : row_off[ci + 1]]
        return seg.rearrange("(p r) v -> p (r v)", p=P, r=r)

    with TileContext(nc) as tc:
        with (
            tc.tile_pool(name="const", bufs=1) as constp,
            tc.tile_pool(name="io", bufs=1) as iop,
            tc.tile_pool(name="work", bufs=1) as workp,
        ):
            cmap_st = constp.tile([P, V], f32, tag="cmap_st")
            iota_st = constp.tile([P, V], f32, tag="iota_st")

            # sync FIFO: chunk 0's x first, tiny consts next, rest of x.
            xts = []
            for ci, r in enumerate(r_list):
                xt = iop.tile([P, r * V], f32, tag=f"x{ci}")
                nc.sync.dma_start(xt[:], chunk_view(x, ci, r))
                xts.append(xt)
                if ci == 0:
                    nc.sync.dma_start(cmap_st[:], cmap[:])
                    nc.sync.dma_start(iota_st[:], iota[:])

            cmap_1 = cmap_st[:].rearrange("p (o v) -> p o v", o=1)
            iota_1 = iota_st[:].rearrange("p (o v) -> p o v", o=1)
            # Table materialization is split: a 16-row slice unblocks the
            # first two (small) chunks fast; the full-size tables are built
            # while the DVE chews on those chunks.
            r_bc = min(16, r_max)
            cmap_f = constp.tile([P, r_max * V], bf16, tag="cmap_f")
            iota_f = constp.tile([P, r_max * V], bf16, tag="iota_f")

            def bcast(dst_tile, src1, lo, hi):
                d3 = dst_tile[:, lo * V : hi * V].rearrange(
                    "p (r v) -> p r v", v=V
                )
                s_b, _ = broadcast_tensor_aps(src1, d3)
                nc.scalar.copy(d3, s_b)

            # Per-chunk op lambdas; each engine's FIFO order is chosen
            # explicitly below (Tile orders by issue-call order per engine).
            adds, reds, pairs, eqs = [], [], [], []
            in_casts, up_casts = {}, []
            yfs = []
            N_F32 = 2   # chunks taking the cast-free f32 add (head chunks)

            def make_chunk(ci):
                r = r_list[ci]
                fd = r * V
                prod = workp.tile([P, fd], bf16, tag=f"prod{ci}")
                p3 = prod[:].rearrange("p (r v) -> p r v", v=V)
                if ci < N_F32:
                    # f32 add straight off the DMA tile + broadcast f32 cmap:
                    # 1x mode, but waits on nothing except the x data. Kills
                    # the in-cast and table dependency on the critical head.
                    x3 = xts[ci][:].rearrange("p (r v) -> p r v", v=V)
                    cmf_b, _ = broadcast_tensor_aps(cmap_1, x3)
                    adds.append(
                        lambda p3=p3, x3=x3, cmf_b=cmf_b: nc.vector.tensor_tensor(
                            p3, x3, cmf_b, op=mybir.AluOpType.add
                        )
                    )
                else:
                    xb_d = workp.tile([P, fd], bf16, tag=f"xb{ci}")
                    in_casts[ci] = lambda xb_d=xb_d, ci=ci: nc.scalar.copy(
                        xb_d[:], xts[ci][:]
                    )
                    adds.append(
                        lambda prod=prod, xb_d=xb_d, fd=fd: nc.vector.tensor_tensor(
                            prod[:], xb_d[:], cmap_f[:, :fd],
                            op=mybir.AluOpType.add,
                        )
                    )
                c_t = workp.tile([P, r], f32, tag=f"c{ci}")
                reds.append(
                    lambda c_t=c_t, p3=p3: nc.vector.tensor_reduce(
                        c_t[:], p3, axis=mybir.AxisListType.X,
                        op=mybir.AluOpType.max,
                    )
                )
                # Duplicate m into adjacent pairs: the is_equal's second
                # operand becomes pair-readable, unlocking TT 2x_1P mode
                # (a plain stride-0 broadcast operand is stuck at 1x).
                m2 = workp.tile([P, 2 * r], bf16, tag=f"m2{ci}")
                m2_3 = m2[:].rearrange("p (r two) -> p r two", two=2)
                c3 = c_t[:].rearrange("p (r one) -> p r one", one=1)
                c3_b, _ = broadcast_tensor_aps(c3, m2_3)
                pairs.append(
                    lambda m2_3=m2_3, c3_b=c3_b: nc.vector.tensor_copy(m2_3, c3_b)
                )
                y_bf = workp.tile([P, fd], bf16, tag=f"ybf{ci}")
                o4 = y_bf[:].rearrange("p (r k two) -> p r k two", two=2, k=V // 2)
                i4 = iota_f[:, :fd].rearrange(
                    "p (r k two) -> p r k two", two=2, k=V // 2
                )
                m2_4 = m2_3.rearrange("p r (one two) -> p r one two", one=1)
                m2_b, _ = broadcast_tensor_aps(m2_4, o4)
                eqs.append(
                    lambda o4=o4, i4=i4, m2_b=m2_b: nc.vector.tensor_tensor(
                        o4, i4, m2_b, op=mybir.AluOpType.is_equal
                    )
                )
                # f32 upcast on scalar (it has slack); out-DMA reads this.
                yf = iop.tile([P, fd], f32, tag=f"yf{ci}")
                up_casts.append(
                    lambda yf=yf, y_bf=y_bf: nc.scalar.copy(yf[:], y_bf[:])
                )
                yfs.append(yf)

            for ci in range(n_chunks):
                make_chunk(ci)

            # Issue order is load-bearing twice over: Tile serializes
            # accesses to a tile by GLOBAL issue order (producers must be
            # issued before consumers), and each engine's FIFO executes in
            # its own issue order. Weave scalar and vector issues so both
            # hold: tables + early in-casts first, then per chunk the DVE
            # chain (pair+eq trailing one chunk) with each upcast issued
            # right after its eq and each later in-cast ahead of its add.
            bcast(cmap_f, cmap_1, 0, r_bc)
            bcast(iota_f, iota_1, 0, r_bc)
            if r_max > r_bc:
                bcast(cmap_f, cmap_1, r_bc, r_max)
            if 2 in in_casts:
                in_casts[2]()
            if r_max > r_bc:
                bcast(iota_f, iota_1, r_bc, r_max)
            if 3 in in_casts:
                in_casts[3]()

            adds[0]()
            reds[0]()
            adds[1]()
            reds[1]()
            for ci in range(2, n_chunks):
                pairs[ci - 2]()
                eqs[ci - 2]()
                up_casts[ci - 2]()
                if ci + 2 in in_casts:
                    in_casts[ci + 2]()
                adds[ci]()
                reds[ci]()
            for ci in range(n_chunks - 2, n_chunks):
                pairs[ci]()
                eqs[ci]()
                up_casts[ci]()

            for ci, r in enumerate(r_list):
                nc.sync.dma_start(chunk_view(y, ci, r), yfs[ci][:])

    # Bacc.finalize runs compile(): wait-splitting (generate_event_semaphores),
    # register allocation, nop fusion. run_bass_via_pjrt serializes nc.m as-is,
    # so this must happen here.
    nc.finalize()
    return nc


def _get_nc(rows_per_core=ROWS_PER_CORE, r_list=R_LIST):
    key = (rows_per_core, tuple(r_list))
    if key not in _CACHE:
        _CACHE[key] = _build_nc(rows_per_core, r_list)
    return _CACHE[key]


def _host_cmap(W: np.ndarray, b: np.ndarray) -> np.ndarray:
    """64-entry map token -> output one-hot index (or sentinel for zero row)."""
    net = W.astype(np.float32) + b.astype(np.float32)[None, :]   # [V, 2V]
    loc_tok = np.argmax(net[:, :V], axis=1)                      # [V]
    scale_tok = np.argmax(net[:, V:], axis=1)                    # [V]
    t = (scale_tok * np.arange(V, dtype=np.int64) + loc_tok) % V
    return np.where(scale_tok == 0, SENTINEL, t.astype(np.float64)).astype(
        np.float32
    )


def _host_tables(W: np.ndarray, b: np.ndarray):
    cmap_eps = _host_cmap(W, b) * np.float32(EPS)                  # exact f32
    iota_eps = 1.0 + np.arange(V, dtype=np.float32) * np.float32(EPS)
    cmap_t = np.tile(cmap_eps.astype(np.float32)[None, :], (P, 1))
    iota_t = np.tile(iota_eps.astype(np.float32)[None, :], (P, 1))
    return cmap_t, iota_t


def kernel(inputs: np.ndarray, W: np.ndarray, b: np.ndarray) -> np.ndarray:
    from concourse import bass_utils

    x = np.ascontiguousarray(inputs.astype(np.float32, copy=False).reshape(ROWS, V))
    cmap_t, iota_t = _host_tables(W, b)

    nc = _get_nc()
    in_maps = [
        {
            "x": x[c * ROWS_PER_CORE : (c + 1) * ROWS_PER_CORE],
            "cmap": cmap_t,
            "iota": iota_t,
        }
        for c in range(N_CORES)
    ]
    res = bass_utils.run_bass_kernel_spmd(nc, in_maps, core_ids=list(range(N_CORES)))
    y = np.concatenate([r["y"] for r in res.results], axis=0)
    return y.reshape(inputs.shape).astype(inputs.dtype, copy=False)
